# revision 26
# baseline (speedup 1.0000x reference)
"""Attention-GRU decoder (teacher forcing) on 8 TRN2 NeuronCores.

v2 strategy (batch-sharded 4 seqs/core, vocab-sharded output):
  Phase 0: precompute EcT (+b1 folded), EncWc, GIX with fp8(x16) weights.
  Phase 1: 31 sequential steps. All recurrence weights (W_hh, W1h, Wc, Wx,
     W1e) are fp8 e4m3 scaled x16 (stationary operands, FWL 4x weight load);
     descale folds into activation `scale` args. sigmoid == 0.5+0.5*tanh(x/2)
     so phase 1 only ever uses Tanh/Exp -> zero act-table thrashing.
     h kept fp16; partial AllGathers (fp16) every 8 steps.
  Phase 2 interleaved: after each partial AllGather, the vocab projection
     (fp8 W_out slice, x16) for those rows runs inside phase-1's tensor
     engine gaps (keeps HAM un-throttled). log-softmax denominators
     AllReduce'd per chunk; finalize (Ln + subtract) deferred/paced.

kernel(**inputs) takes full inputs, returns [B, T-1, V] float32.
"""
import numpy as np
import ml_dtypes

import concourse.bacc as bacc
import concourse.bass as bass
import concourse.bass_utils as _bu
import concourse.mybir as mybir
import concourse.tile as tile
from concourse.bass_utils import run_bass_kernel_spmd

del _bu  # (ldw-opt experiment removed: incompatible with K=1 ldweights)

F32 = mybir.dt.float32
F16 = mybir.dt.float16
F8 = mybir.dt.float8e4
AF = mybir.ActivationFunctionType
ALU = mybir.AluOpType

B, S, H, V, Dw, T = 32, 50, 1024, 32000, 512, 32
NCORES = 8
P = 128
TS = T - 1            # 31 decode steps
BC = B // NCORES      # 4 sequences per core
VC = V // NCORES      # 4000 vocab rows per core
SP = 64               # padded s-block per sequence
NBS = BC * SP         # 256 padded (b,s) columns per core
ROWS = TS * BC        # 124 hidden rows per rank
RTOT = TS * B         # 992 total rows
KH = H // P           # 8 hidden chunks
KG = 3 * H // P       # 24 gate chunks
NV = 8                # vocab n-chunks per core
NVS = VC // NV        # 500
SW = 16.0             # fp8 weight scale
ISW = 1.0 / SW
AG_CHUNKS = [(1, 9), (9, 17), (17, 25), (25, 32)]
CW = [(thi - tlo) * BC for (tlo, thi) in AG_CHUNKS]   # 32,32,32,28

_CACHE = {}


def _build():
    nc = bacc.Bacc("TRN2", target_bir_lowering=False, debug=False,
                   num_devices=NCORES)

    def din(name, shape, dt):
        return nc.dram_tensor(name, shape, dt, kind="ExternalInput").ap()

    enct16_d = din("enct16", [P, KH, NBS], F16)
    w1et8_d = din("w1et8", [P, KH, H], F8)
    wct8_d = din("wct8", [P, KH, 3 * H], F8)
    wxat8_d = din("wxat8", [P, 5, 3 * H], F8)
    xat16_d = din("xat16", [P, 5, P], F16)
    whht8_d = din("whht8", [P, KH, 3 * H], F8)
    w1ht8_d = din("w1ht8", [P, KH, H], F8)
    w2t16_d = din("w2t16", [P, KH], F16)
    b1t_d = din("b1t", [P, KH], F32)
    bhnrep_d = din("bhnrep", [P, KH * BC], F32)
    h0t_d = din("h0t", [P, KH * BC], F16)
    woutt8_d = din("woutt8", [P, KH, VC], F8)
    bout16_d = din("bout16", [1, VC], F16)
    out_d = nc.dram_tensor("out", [RTOT, VC], F32, kind="ExternalOutput").ap()

    rg = [list(range(NCORES))]
    outv = out_d.rearrange("(m tb) v -> m tb v", tb=ROWS)

    with tile.TileContext(nc) as tc:
        with tc.tile_pool(name="dram", bufs=1, space="DRAM") as dram:
            agin, agout, arin, arout = [], [], [], []
            for j, (tlo, thi) in enumerate(AG_CHUNKS):
                w = CW[j]
                agin.append(dram.tile([H, w], F16, name=f"agin{j}"))
                agout.append(dram.tile([NCORES, H, w], F16, name=f"agout{j}"))
                arin.append(dram.tile([2 * P, 1], F32, name=f"arin{j}"))
                arout.append(dram.tile([2 * P, 1], F32, name=f"arout{j}"))

            pwo_cm = tc.tile_pool(name="pwo", bufs=1)
            pwo = pwo_cm.__enter__()
            wo8 = pwo.tile([P, KH, VC], F8)
            hgat16 = pwo.tile([P, KH, 2, 4, 4, 32], F16)
            boutrep = pwo.tile([P, VC], F16)
            lg_t = [[pwo.tile([P, VC], F16, name=f"lg{j}_{h_}")
                     for h_ in range(2)] for j in range(4)]
            sums_t = [[pwo.tile([P, NV], F32, name=f"sums{j}_{h_}")
                       for h_ in range(2)] for j in range(4)]

            with tc.tile_pool(name="pw", bufs=1) as pw:
                # ---- persistent phase-0/1 tiles ----
                whht8 = pw.tile([P, KH, 3 * H], F8)
                w1ht8 = pw.tile([P, KH, H], F8)
                ecT16 = pw.tile([P, KH, NBS], F16)
                encwc8 = pw.tile([P, 2, 3 * H], F8)
                gixt = pw.tile([P, KG, TS, BC], F16)
                hallT = pw.tile([P, KH, T, BC], F16)
                aw16 = pw.tile([P, KH, NBS], F16)
                w2t16 = pw.tile([P, KH], F16)
                bhnrep = pw.tile([P, KH, BC], F32)
                ones1 = pw.tile([1, 1], F16)
                bd1 = pw.tile([P, BC], F16)
                bd2 = pw.tile([P, BC], F16)

                nc.sync.dma_start(out=w2t16[:], in_=w2t16_d[:])
                nc.sync.dma_start(
                    out=bhnrep[:],
                    in_=bhnrep_d[:].rearrange("p (k b) -> p k b", b=BC))
                nc.sync.dma_start(
                    out=hallT[:, :, 0, :],
                    in_=h0t_d[:].rearrange("p (k b) -> p k b", b=BC))
                nc.vector.memset(ones1[:], 1.0)
                nc.vector.memset(bd1[:], 0.0)
                nc.vector.memset(bd2[:], 0.0)
                nc.vector.memset(aw16[:], 0.0)
                for k in range(KH):
                    nc.vector.memset(hgat16[:, k, :, 3, :, 28:32], 0.0)

                # ---------------- phase 0 ----------------
                with (
                    tc.tile_pool(name="p0a", bufs=1) as p0a,
                    tc.tile_pool(name="p0as", bufs=2) as p0as,
                    tc.tile_pool(name="ps_gx_pool", bufs=1, space="PSUM") as psgx,
                    tc.tile_pool(name="ps_bo_pool", bufs=1, space="PSUM") as psbo,
                ):
                    # b_out broadcast to all partitions (needed in phase 1)
                    ones16 = p0a.tile([1, P], F16)
                    nc.vector.memset(ones16[:], 1.0)
                    bout16 = p0a.tile([1, VC], F16)
                    nc.sync.dma_start(out=bout16[:], in_=bout16_d[:])
                    for n in range(NV):
                        bsl = slice(n * NVS, (n + 1) * NVS)
                        ps_b = psbo.tile([P, NVS], F32, name="ps_b", tag="ps_b")
                        nc.tensor.matmul(ps_b[:], ones16[:], bout16[:, bsl],
                                         start=True, stop=True)
                        nc.scalar.copy(boutrep[:, bsl], ps_b[:])

                    xat16 = p0a.tile([P, 5, P], F16)
                    nc.sync.dma_start(out=xat16[:], in_=xat16_d[:])
                    ps_gx = [psgx.tile([P, 4, P], F32, name=f"ps_gx{g}")
                             for g in range(6)]
                    for k in range(5):
                        wxk = p0as.tile([P, 3 * H], F8, name="wxk", tag="wxk")
                        nc.sync.dma_start(out=wxk[:], in_=wxat8_d[:, k, :])
                        for mo in range(KG):
                            nc.tensor.matmul(
                                ps_gx[mo // 4][:, mo % 4, :],
                                wxk[:, mo * P:(mo + 1) * P],
                                xat16[:, k, :], start=(k == 0), stop=(k == 4))
                    for mo in range(KG):
                        nc.scalar.copy(
                            gixt[:, mo, :, :],
                            ps_gx[mo // 4][:, mo % 4, 0:ROWS].rearrange(
                                "p (t b) -> p t b", b=BC))

                with (
                    tc.tile_pool(name="p0b", bufs=1) as p0b,
                    tc.tile_pool(name="p0bs", bufs=2) as p0bs,
                ):
                    enct16 = p0b.tile([P, KH, NBS], F16)
                    b1t = p0b.tile([P, KH], F32)
                    nc.sync.dma_start(out=b1t[:], in_=b1t_d[:])
                    nc.sync.dma_start(out=enct16[:], in_=enct16_d[:])

                    # EcT (+ b1 folded), fp16 out
                    with tc.tile_pool(name="ps_ec_pool", bufs=1,
                                      space="PSUM") as psec:
                        ps_ec = [psec.tile([P, NBS], F32, name=f"ps_ec{mo}")
                                 for mo in range(KH)]
                        for k in range(KH):
                            w1ek = p0bs.tile([P, H], F8, name="w1ek", tag="w1ek")
                            nc.sync.dma_start(out=w1ek[:], in_=w1et8_d[:, k, :])
                            for mo in range(KH):
                                nc.tensor.matmul(
                                    ps_ec[mo][:], w1ek[:, mo * P:(mo + 1) * P],
                                    enct16[:, k, :],
                                    start=(k == 0), stop=(k == KH - 1))
                        for mo in range(KH):
                            nc.vector.tensor_scalar(
                                ecT16[:, mo, :], ps_ec[mo][:],
                                b1t[:, mo:mo + 1], None, op0=ALU.add)

                    # EncWc, fp8 out (x16 scale via host-scaled wct8)
                    with tc.tile_pool(name="ps_ew_pool", bufs=2,
                                      space="PSUM") as psew:
                        for n in range(12):
                            wcs = p0bs.tile([P, KH, 256], F8, name="wcs",
                                            tag="wcs")
                            nc.sync.dma_start(
                                out=wcs[:],
                                in_=wct8_d[:, :, n * 256:(n + 1) * 256])
                            for mt in range(2):
                                ps_ew = psew.tile([P, 256], F32, name="ps_ew",
                                                  tag="ps_ew")
                                for k in range(KH):
                                    nc.tensor.matmul(
                                        ps_ew[:],
                                        enct16[:, k, mt * P:(mt + 1) * P],
                                        wcs[:, k, :],
                                        start=(k == 0), stop=(k == KH - 1))
                                nc.vector.tensor_copy(
                                    encwc8[:, mt, n * 256:(n + 1) * 256],
                                    ps_ew[:])

                nc.sync.dma_start(out=whht8[:], in_=whht8_d[:])
                nc.sync.dma_start(out=w1ht8[:], in_=w1ht8_d[:])
                nc.sync.dma_start(out=wo8[:], in_=woutt8_d[:])

                # ---------------- phase 1 + interleaved phase 2 ----------
                p2_queue = []     # pending (j, half, n) vocab chunks
                p2_done = [0] * 4
                fin_at = {20: [0], 28: [1]}

                with (
                    tc.tile_pool(name="p1", bufs=2) as p1,
                    tc.tile_pool(name="p2x", bufs=3) as p2x,
                    tc.tile_pool(name="ps_hp_pool", bufs=1, space="PSUM") as pshp,
                    tc.tile_pool(name="ps_gh_pool", bufs=1, space="PSUM") as psgh,
                    tc.tile_pool(name="ps_gic_pool", bufs=1, space="PSUM") as psgic,
                    tc.tile_pool(name="ps_e_pool", bufs=1, space="PSUM") as pse,
                    tc.tile_pool(name="ps_a_pool", bufs=1, space="PSUM") as psa,
                    tc.tile_pool(name="ps_o_pool", bufs=2, space="PSUM") as pso,
                ):
                    def p2_chunk(j, half, n):
                        w = CW[j]
                        rows = 4 * w
                        nsl = slice(n * NVS, (n + 1) * NVS)
                        ps_o = pso.tile([P, NVS], F32, name="ps_o", tag="ps_o")
                        for k in range(KH):
                            nc.tensor.matmul(
                                ps_o[:],
                                hgat16[:, k, half, j, :, :].rearrange(
                                    "p m c -> p (m c)"),
                                wo8[:, k, nsl],
                                start=(k == 0), stop=(k == KH - 1))
                        lg = lg_t[j][half]
                        nc.vector.scalar_tensor_tensor(
                            lg[:, nsl], ps_o[:], ISW,
                            boutrep[:, nsl], op0=ALU.mult, op1=ALU.add)
                        etr = p2x.tile([P, NVS], F16, name="etr", tag="etr")
                        nc.scalar.activation(
                            etr[:], lg[:, nsl], AF.Exp,
                            accum_out=sums_t[j][half][:, n:n + 1])
                        p2_done[j] += 1
                        if n == NV - 1:
                            ssum = p2x.tile([P, 1], F32, name="ssum",
                                            tag="ssum")
                            nc.vector.reduce_sum(
                                ssum[:], sums_t[j][half][:],
                                axis=mybir.AxisListType.X)
                            nc.sync.dma_start(
                                out=arin[j][half * P:(half + 1) * P, :],
                                in_=ssum[:])
                        if p2_done[j] == 2 * NV:
                            nc.gpsimd.collective_compute(
                                "AllReduce", ALU.add, replica_groups=rg,
                                ins=[arin[j].opt()], outs=[arout[j].opt()])

                    def finalize(j):
                        (tlo, thi) = AG_CHUNKS[j]
                        w = CW[j]
                        for h_ in range(2):
                            lzt = p2x.tile([P, 1], F32, name="lzt", tag="lzt")
                            nc.sync.dma_start(
                                out=lzt[:],
                                in_=arout[j][h_ * P:(h_ + 1) * P, :])
                            lzl = p2x.tile([P, 1], F32, name="lzl", tag="lzl")
                            nc.scalar.activation(lzl[:], lzt[:], AF.Ln)
                            nlz = p2x.tile([P, 1], F32, name="nlz", tag="nlz")
                            nc.vector.tensor_scalar(
                                nlz[:], lzl[:], -1.0, None, op0=ALU.mult)
                            lg = lg_t[j][h_]
                            half_v = VC // 4
                            for vv in range(4):
                                vsl = slice(vv * half_v, (vv + 1) * half_v)
                                ost = p2x.tile([P, half_v], F32, name="ost",
                                               tag="ost", bufs=3)
                                nc.vector.tensor_scalar(
                                    ost[:], lg[:, vsl],
                                    lzl[:, 0:1], None,
                                    op0=ALU.subtract)
                                for m in range(4):
                                    r0 = (h_ * 4 + m) * ROWS + (tlo - 1) * BC
                                    nc.gpsimd.dma_start(
                                        out=out_d[r0:r0 + w, vsl],
                                        in_=ost[m * 32:m * 32 + w, :])

                    for t in range(1, T):
                        hprev = hallT[:, :, t - 1, :]

                        # hproj (fp8 W1h, x16)
                        ps_hp = pshp.tile([P, KH, BC], F32, name="ps_hp",
                                          tag="hp")
                        for mo in range(KH):
                            for k in range(KH):
                                nc.tensor.matmul(
                                    ps_hp[:, mo, :],
                                    w1ht8[:, k, mo * P:(mo + 1) * P],
                                    hprev[:, k, :],
                                    start=(k == 0), stop=(k == KH - 1))

                        # attention: aw = tanh((EcT16 + Hproj)/16), e = w2.aw
                        ps_e = pse.tile([1, NBS], F32, name="ps_e", tag="e")
                        for g in range(4):
                            msl = slice(2 * g, 2 * g + 2)
                            awp = p1.tile([P, 2, BC, SP], F16, name="awp",
                                          tag="awp")
                            nc.vector.tensor_add(
                                awp[:],
                                ps_hp[:, msl, :].broadcast_to([P, 2, BC, SP]),
                                ecT16[:, msl, :].rearrange(
                                    "p m (b s) -> p m b s", s=SP))
                            nc.scalar.activation(
                                aw16[:, msl, :].rearrange(
                                    "p m (b s) -> p m b s", s=SP),
                                awp[:], AF.Tanh, scale=ISW)
                            for mo in range(2 * g, 2 * g + 2):
                                nc.tensor.matmul(
                                    ps_e[:], w2t16[:, mo:mo + 1],
                                    aw16[:, mo, :],
                                    start=(mo == 0), stop=(mo == KH - 1))

                        # gh (fp8 W_hh, x16) — after the attention matmuls
                        # so the e-reduction isn't queued behind 192 MMs
                        ps_gh = psgh.tile([P, KG, BC], F32, name="ps_gh",
                                          tag="gh")
                        for mo in range(KG):
                            for k in range(KH):
                                nc.tensor.matmul(
                                    ps_gh[:, mo, :],
                                    whht8[:, k, mo * P:(mo + 1) * P],
                                    hprev[:, k, :],
                                    start=(k == 0), stop=(k == KH - 1))

                        # softmax over s (no max-shift: |e| small)
                        expe = p1.tile([1, NBS], F32, name="expe", tag="expe")
                        nc.scalar.activation(expe[:], ps_e[:], AF.Exp)
                        s4 = p1.tile([1, BC], F32, name="s4", tag="s4")
                        nc.vector.reduce_sum(
                            s4[:], expe[:].rearrange("a (b s) -> a b s", s=SP)
                            [:, :, 0:S],
                            axis=mybir.AxisListType.X)
                        r4 = p1.tile([1, BC], F32, name="r4", tag="r4")
                        nc.vector.reciprocal(r4[:], s4[:])
                        alphan = p1.tile([1, NBS], F16, name="alphan",
                                         tag="aln")
                        av = alphan[:].rearrange("a (b s) -> a b s", s=SP)
                        nc.vector.tensor_mul(
                            av, expe[:].rearrange("a (b s) -> a b s", s=SP),
                            r4[:].broadcast_to([1, BC, SP]))
                        nc.vector.memset(
                            alphan[:].rearrange(
                                "a (b s) -> a b s", s=SP)[:, :, S:SP], 0.0)

                        # transpose alpha to partitions via K=1 matmuls
                        ps_a = psa.tile([P, 2], F32, name="ps_a", tag="a")
                        nc.tensor.matmul(ps_a[:, 0:1], alphan[:, 0:P],
                                         ones1[:], start=True, stop=True)
                        nc.tensor.matmul(ps_a[:, 1:2], alphan[:, P:NBS],
                                         ones1[:], start=True, stop=True)
                        nc.vector.tensor_copy(bd1[0:64, 0:1], ps_a[0:64, 0:1])
                        nc.vector.tensor_copy(bd1[64:128, 1:2],
                                              ps_a[64:128, 0:1])
                        nc.vector.tensor_copy(bd2[0:64, 2:3], ps_a[0:64, 1:2])
                        nc.vector.tensor_copy(bd2[64:128, 3:4],
                                              ps_a[64:128, 1:2])

                        # gi_c = blockdiag(alpha) applied to EncWc (fp8, x16)
                        ps_gic = psgic.tile([P, KG, BC], F32, name="ps_gic",
                                            tag="gic")
                        for mo in range(KG):
                            nc.tensor.matmul(
                                ps_gic[:, mo, :],
                                encwc8[:, 0, mo * P:(mo + 1) * P],
                                bd1[:], start=True, stop=False)
                            nc.tensor.matmul(
                                ps_gic[:, mo, :],
                                encwc8[:, 1, mo * P:(mo + 1) * P],
                                bd2[:], start=False, stop=True)

                        # gates (all pre-activations carry x16 scale;
                        # sigmoid(x) = 0.5 + 0.5*tanh(x/2) -> Tanh only)
                        s1 = p1.tile([P, KG, BC], F16, name="s1", tag="s1")
                        nc.vector.tensor_add(s1[:], ps_gic[:],
                                             gixt[:, :, t - 1, :])
                        srz = p1.tile([P, 2 * KH, BC], F16, name="srz",
                                      tag="srz")
                        nc.vector.tensor_add(srz[:], s1[:, 0:2 * KH, :],
                                             ps_gh[:, 0:2 * KH, :])
                        rzt = p1.tile([P, 2 * KH, BC], F16, name="rzt",
                                      tag="rzt")
                        nc.scalar.activation(rzt[:], srz[:], AF.Tanh,
                                             scale=0.5 * ISW)
                        rz = p1.tile([P, 2 * KH, BC], F16, name="rz", tag="rz")
                        nc.vector.tensor_scalar(rz[:], rzt[:], 0.5, 0.5,
                                                op0=ALU.mult, op1=ALU.add)
                        hn = p1.tile([P, KH, BC], F16, name="hn", tag="hn")
                        nc.vector.tensor_add(hn[:], ps_gh[:, 2 * KH:KG, :],
                                             bhnrep[:])
                        m1 = p1.tile([P, KH, BC], F16, name="m1", tag="m1")
                        nc.vector.tensor_mul(m1[:], rz[:, 0:KH, :], hn[:])
                        s3 = p1.tile([P, KH, BC], F16, name="s3", tag="s3")
                        nc.vector.tensor_add(s3[:], s1[:, 2 * KH:KG, :], m1[:])
                        nn_t = p1.tile([P, KH, BC], F16, name="nn_t", tag="nn")
                        nc.scalar.activation(nn_t[:], s3[:], AF.Tanh,
                                             scale=ISW)
                        dd = p1.tile([P, KH, BC], F16, name="dd", tag="dd")
                        nc.vector.tensor_sub(dd[:], hprev, nn_t[:])
                        m2 = p1.tile([P, KH, BC], F16, name="m2", tag="m2")
                        nc.vector.tensor_mul(m2[:], rz[:, KH:2 * KH, :], dd[:])
                        nc.vector.tensor_add(hallT[:, :, t, :], nn_t[:], m2[:])

                        # partial allgather of finished h slots (fp16)
                        for j, (tlo, thi) in enumerate(AG_CHUNKS):
                            if t == thi - 1:
                                w = CW[j]
                                for k in range(KH):
                                    nc.sync.dma_start(
                                        out=agin[j][k * P:(k + 1) * P, :]
                                            .rearrange("p (t b) -> p t b",
                                                       b=BC),
                                        in_=hallT[:, k, tlo:thi, :])
                                nc.gpsimd.collective_compute(
                                    "AllGather", ALU.bypass, replica_groups=rg,
                                    ins=[agin[j].opt()], outs=[agout[j].opt()])
                                for k in range(KH):
                                    hgs = p1.tile([P, NCORES, 32], F16,
                                                  name="hgs", tag="hgs")
                                    nc.sync.dma_start(
                                        out=hgs[:, :, 0:w],
                                        in_=agout[j][:, k * P:(k + 1) * P, :]
                                            .rearrange("r p w -> p r w"))
                                    nc.vector.tensor_copy(
                                        hgat16[:, k, :, j, :, 0:w],
                                        hgs[:, :, 0:w].rearrange(
                                            "p (x m) w -> p x m w", x=2))
                                for half in range(2):
                                    for n in range(NV):
                                        p2_queue.append((j, half, n))

                        # paced phase-2 interleave: 2 chunks per step
                        for _ in range(2):
                            if p2_queue:
                                p2_chunk(*p2_queue.pop(0))
                        for j in fin_at.get(t, []):
                            finalize(j)

                    # tail: remaining vocab chunks + finalizes
                    while p2_queue:
                        p2_chunk(*p2_queue.pop(0))
                    finalize(2)
                    finalize(3)

            pwo_cm.__exit__(None, None, None)

    nc.compile()
    return nc


def _t8(w, nk=8):
    # [nk*128, M] -> [128, nk, M]
    m = w.shape[1]
    return np.ascontiguousarray(w.reshape(nk, P, m).transpose(1, 0, 2))


def _f8(x):
    return np.ascontiguousarray(np.asarray(x * SW, dtype=np.float32)).astype(
        ml_dtypes.float8_e4m3)


def _prep_inputs(inputs):
    enc = np.asarray(inputs["encoder_outputs"], np.float32)
    ehid = np.asarray(inputs["encoder_hidden"], np.float32)
    targets = np.asarray(inputs["targets"])
    emb = np.asarray(inputs["emb"], np.float32)
    W1 = np.asarray(inputs["attn_W1"], np.float32)
    b1 = np.asarray(inputs["attn_b1"], np.float32)
    W2 = np.asarray(inputs["attn_W2"], np.float32)
    W_ih = np.asarray(inputs["W_ih"], np.float32)
    b_ih = np.asarray(inputs["b_ih"], np.float32)
    W_hh = np.asarray(inputs["W_hh"], np.float32)
    b_hh = np.asarray(inputs["b_hh"], np.float32)
    W_out = np.asarray(inputs["W_out"], np.float32)
    b_out = np.asarray(inputs["b_out"], np.float32)

    # shared (replicated across cores); all recurrence weights fp8 e4m3 x16
    w1et8 = _f8(_t8(W1[:, :H].T))
    w1ht8 = _f8(_t8(np.ascontiguousarray(W1[:, H:]).T))
    wct8 = _f8(_t8(np.ascontiguousarray(W_ih[:, Dw:]).T))
    whht8 = _f8(_t8(W_hh.T))
    wxa = np.zeros((640, 3 * H), np.float32)
    wxa[:Dw] = W_ih[:, :Dw].T
    wxa[Dw] = b_ih + np.concatenate([b_hh[:2 * H], np.zeros(H, np.float32)])
    wxat8 = _f8(_t8(wxa, nk=5))
    w2t16 = np.ascontiguousarray(W2[0].reshape(KH, P).T).astype(np.float16)
    b1t = np.ascontiguousarray(b1.reshape(KH, P).T) * SW
    bhnrep = np.ascontiguousarray(
        np.repeat(b_hh[2 * H:].reshape(KH, P).T[:, :, None], BC, axis=2)
        .reshape(P, KH * BC)) * SW

    x_all = emb[targets[:, :TS]]  # [B, TS, Dw]

    in_maps = []
    for c in range(NCORES):
        bsl = slice(c * BC, (c + 1) * BC)
        vsl = slice(c * VC, (c + 1) * VC)
        encT = np.zeros((H, BC, SP), np.float32)
        encT[:, :, :S] = enc[bsl].transpose(2, 0, 1)
        enct16 = _t8(encT.reshape(H, NBS)).astype(np.float16)
        xat = np.zeros((640, P), np.float32)
        xat[:Dw, :ROWS] = x_all[bsl].transpose(2, 1, 0).reshape(Dw, ROWS)
        xat[Dw, :ROWS] = 1.0
        xat16 = _t8(xat, nk=5).astype(np.float16)
        h0t = np.ascontiguousarray(
            ehid[0, bsl].T.reshape(KH, P, BC).transpose(1, 0, 2)
            .reshape(P, KH * BC)).astype(np.float16)
        woutt8 = _f8(_t8(np.ascontiguousarray(W_out[vsl]).T))
        bout16 = np.ascontiguousarray(b_out[vsl][None, :]).astype(np.float16)
        in_maps.append({
            "enct16": enct16, "w1et8": w1et8, "wct8": wct8,
            "wxat8": wxat8, "xat16": xat16, "whht8": whht8, "w1ht8": w1ht8,
            "w2t16": w2t16, "b1t": b1t, "bhnrep": bhnrep, "h0t": h0t,
            "woutt8": woutt8, "bout16": bout16,
        })
    return in_maps


def kernel(**inputs):
    if "nc" not in _CACHE:
        _CACHE["nc"] = _build()
    nc = _CACHE["nc"]
    in_maps = _prep_inputs(inputs)
    res = run_bass_kernel_spmd(nc, in_maps, core_ids=list(range(NCORES)))
    L = np.stack([res.results[c]["out"] for c in range(NCORES)])
    L = (L.reshape(NCORES, NCORES, TS, BC, VC)
         .transpose(1, 3, 2, 0, 4).reshape(B, TS, V))
    return np.ascontiguousarray(L, dtype=np.float32)


# revision 28
# speedup vs baseline: 1.1310x; 1.1310x over previous
"""Attention-GRU decoder (teacher forcing) on 8 TRN2 NeuronCores.

v2 strategy (batch-sharded 4 seqs/core, vocab-sharded output):
  Phase 0: precompute EcT (+b1 folded), EncWc, GIX with fp8(x16) weights.
  Phase 1: 31 sequential steps. All recurrence weights (W_hh, W1h, Wc, Wx,
     W1e) are fp8 e4m3 scaled x16 (stationary operands, FWL 4x weight load);
     descale folds into activation `scale` args. sigmoid == 0.5+0.5*tanh(x/2)
     so phase 1 only ever uses Tanh/Exp -> zero act-table thrashing.
     h kept fp16; partial AllGathers (fp16) every 8 steps.
  Phase 2 interleaved: after each partial AllGather, the vocab projection
     (fp8 W_out slice, x16) for those rows runs inside phase-1's tensor
     engine gaps (keeps HAM un-throttled). log-softmax denominators
     AllReduce'd per chunk; finalize (Ln + subtract) deferred/paced.

kernel(**inputs) takes full inputs, returns [B, T-1, V] float32.
"""
import numpy as np
import ml_dtypes

import concourse.bacc as bacc
import concourse.bass as bass
import concourse.bass_utils as _bu
import concourse.mybir as mybir
import concourse.tile as tile
from concourse.bass_utils import run_bass_kernel_spmd

del _bu  # (ldw-opt experiment removed: incompatible with K=1 ldweights)

F32 = mybir.dt.float32
F16 = mybir.dt.float16
F8 = mybir.dt.float8e4
AF = mybir.ActivationFunctionType
ALU = mybir.AluOpType

B, S, H, V, Dw, T = 32, 50, 1024, 32000, 512, 32
NCORES = 8
P = 128
TS = T - 1            # 31 decode steps
BC = B // NCORES      # 4 sequences per core
VC = V // NCORES      # 4000 vocab rows per core
SP = 64               # padded s-block per sequence
NBS = BC * SP         # 256 padded (b,s) columns per core
ROWS = TS * BC        # 124 hidden rows per rank
RTOT = TS * B         # 992 total rows
KH = H // P           # 8 hidden chunks
KG = 3 * H // P       # 24 gate chunks
NV = 8                # vocab n-chunks per core
NVS = VC // NV        # 500
SW = 16.0             # fp8 weight scale
ISW = 1.0 / SW
AG_CHUNKS = [(1, 9), (9, 17), (17, 25), (25, 32)]
CW = [(thi - tlo) * BC for (tlo, thi) in AG_CHUNKS]   # 32,32,32,28

_CACHE = {}


def _build():
    nc = bacc.Bacc("TRN2", target_bir_lowering=False, debug=False,
                   num_devices=NCORES)

    def din(name, shape, dt):
        return nc.dram_tensor(name, shape, dt, kind="ExternalInput").ap()

    enct16_d = din("enct16", [P, KH, NBS], F16)
    w1et8_d = din("w1et8", [P, KH, H], F8)
    wct8_d = din("wct8", [P, KH, 3 * H], F8)
    wxat8_d = din("wxat8", [P, 5, 3 * H], F8)
    xat16_d = din("xat16", [P, 5, P], F16)
    whht8_d = din("whht8", [P, KH, 3 * H], F8)
    w1ht8_d = din("w1ht8", [P, KH, H], F8)
    w2t16_d = din("w2t16", [P, KH], F16)
    b1t_d = din("b1t", [P, KH], F32)
    bhnrep_d = din("bhnrep", [P, KH * BC], F32)
    h0t_d = din("h0t", [P, KH * BC], F16)
    woutt8_d = din("woutt8", [P, KH, VC], F8)
    bout16_d = din("bout16", [1, VC], F16)
    out_d = nc.dram_tensor("out", [RTOT, VC], F32, kind="ExternalOutput").ap()

    rg = [list(range(NCORES))]
    outv = out_d.rearrange("(m tb) v -> m tb v", tb=ROWS)

    with tile.TileContext(nc) as tc:
        with tc.tile_pool(name="dram", bufs=1, space="DRAM") as dram:
            agin, agout, arin, arout = [], [], [], []
            for j, (tlo, thi) in enumerate(AG_CHUNKS):
                w = CW[j]
                agin.append(dram.tile([H, w], F16, name=f"agin{j}"))
                agout.append(dram.tile([NCORES, H, w], F16, name=f"agout{j}"))
                arin.append(dram.tile([2 * P, 1], F32, name=f"arin{j}"))
                arout.append(dram.tile([2 * P, 1], F32, name=f"arout{j}"))

            pwo_cm = tc.tile_pool(name="pwo", bufs=1)
            pwo = pwo_cm.__enter__()
            wo8 = pwo.tile([P, KH, VC], F8)
            hgat16 = pwo.tile([P, KH, 2, 4, 4, 32], F16)
            boutrep = pwo.tile([P, VC], F16)
            lg_t = [[pwo.tile([P, VC], F16, name=f"lg{j}_{h_}")
                     for h_ in range(2)] for j in range(4)]
            sums_t = [[pwo.tile([P, NV], F32, name=f"sums{j}_{h_}")
                       for h_ in range(2)] for j in range(4)]

            with tc.tile_pool(name="pw", bufs=1) as pw:
                # ---- persistent phase-0/1 tiles ----
                whht8 = pw.tile([P, KH, 3 * H], F8)
                w1ht8 = pw.tile([P, KH, H], F8)
                ecT16 = pw.tile([P, KH, NBS], F16)
                encwc8 = pw.tile([P, 2, 3 * H], F8)
                gixt = pw.tile([P, KG, TS, BC], F16)
                hallT = pw.tile([P, KH, T, BC], F16)
                aw16 = pw.tile([P, KH, NBS], F16)
                w2t16 = pw.tile([P, KH], F16)
                bhnrep = pw.tile([P, KH, BC], F32)
                ones1 = pw.tile([1, 1], F16)
                bd1 = pw.tile([P, BC], F16)
                bd2 = pw.tile([P, BC], F16)

                nc.sync.dma_start(out=w2t16[:], in_=w2t16_d[:])
                nc.sync.dma_start(
                    out=bhnrep[:],
                    in_=bhnrep_d[:].rearrange("p (k b) -> p k b", b=BC))
                nc.sync.dma_start(
                    out=hallT[:, :, 0, :],
                    in_=h0t_d[:].rearrange("p (k b) -> p k b", b=BC))
                nc.vector.memset(ones1[:], 1.0)
                nc.vector.memset(bd1[:], 0.0)
                nc.vector.memset(bd2[:], 0.0)
                nc.vector.memset(aw16[:], 0.0)
                for k in range(KH):
                    nc.vector.memset(hgat16[:, k, :, 3, :, 28:32], 0.0)

                # ---------------- phase 0 ----------------
                with (
                    tc.tile_pool(name="p0a", bufs=1) as p0a,
                    tc.tile_pool(name="p0as", bufs=2) as p0as,
                    tc.tile_pool(name="ps_gx_pool", bufs=1, space="PSUM") as psgx,
                    tc.tile_pool(name="ps_bo_pool", bufs=1, space="PSUM") as psbo,
                ):
                    # b_out broadcast to all partitions (needed in phase 1)
                    ones16 = p0a.tile([1, P], F16)
                    nc.vector.memset(ones16[:], 1.0)
                    bout16 = p0a.tile([1, VC], F16)
                    nc.sync.dma_start(out=bout16[:], in_=bout16_d[:])
                    for n in range(NV):
                        bsl = slice(n * NVS, (n + 1) * NVS)
                        ps_b = psbo.tile([P, NVS], F32, name="ps_b", tag="ps_b")
                        nc.tensor.matmul(ps_b[:], ones16[:], bout16[:, bsl],
                                         start=True, stop=True)
                        nc.scalar.copy(boutrep[:, bsl], ps_b[:])

                    xat16 = p0a.tile([P, 5, P], F16)
                    nc.sync.dma_start(out=xat16[:], in_=xat16_d[:])
                    ps_gx = [psgx.tile([P, 4, P], F32, name=f"ps_gx{g}")
                             for g in range(6)]
                    for k in range(5):
                        wxk = p0as.tile([P, 3 * H], F8, name="wxk", tag="wxk")
                        nc.sync.dma_start(out=wxk[:], in_=wxat8_d[:, k, :])
                        for mo in range(KG):
                            nc.tensor.matmul(
                                ps_gx[mo // 4][:, mo % 4, :],
                                wxk[:, mo * P:(mo + 1) * P],
                                xat16[:, k, :], start=(k == 0), stop=(k == 4))
                    for mo in range(KG):
                        nc.scalar.copy(
                            gixt[:, mo, :, :],
                            ps_gx[mo // 4][:, mo % 4, 0:ROWS].rearrange(
                                "p (t b) -> p t b", b=BC))

                with (
                    tc.tile_pool(name="p0b", bufs=1) as p0b,
                    tc.tile_pool(name="p0bs", bufs=2) as p0bs,
                ):
                    enct16 = p0b.tile([P, KH, NBS], F16)
                    b1t = p0b.tile([P, KH], F32)
                    nc.sync.dma_start(out=b1t[:], in_=b1t_d[:])
                    nc.sync.dma_start(out=enct16[:], in_=enct16_d[:])

                    # EcT (+ b1 folded), fp16 out
                    with tc.tile_pool(name="ps_ec_pool", bufs=1,
                                      space="PSUM") as psec:
                        ps_ec = [psec.tile([P, NBS], F32, name=f"ps_ec{mo}")
                                 for mo in range(KH)]
                        for k in range(KH):
                            w1ek = p0bs.tile([P, H], F8, name="w1ek", tag="w1ek")
                            nc.sync.dma_start(out=w1ek[:], in_=w1et8_d[:, k, :])
                            for mo in range(KH):
                                nc.tensor.matmul(
                                    ps_ec[mo][:], w1ek[:, mo * P:(mo + 1) * P],
                                    enct16[:, k, :],
                                    start=(k == 0), stop=(k == KH - 1))
                        for mo in range(KH):
                            nc.vector.tensor_scalar(
                                ecT16[:, mo, :], ps_ec[mo][:],
                                b1t[:, mo:mo + 1], None, op0=ALU.add)

                    # EncWc, fp8 out (x16 scale via host-scaled wct8)
                    with tc.tile_pool(name="ps_ew_pool", bufs=2,
                                      space="PSUM") as psew:
                        for n in range(12):
                            wcs = p0bs.tile([P, KH, 256], F8, name="wcs",
                                            tag="wcs")
                            nc.sync.dma_start(
                                out=wcs[:],
                                in_=wct8_d[:, :, n * 256:(n + 1) * 256])
                            for mt in range(2):
                                ps_ew = psew.tile([P, 256], F32, name="ps_ew",
                                                  tag="ps_ew")
                                for k in range(KH):
                                    nc.tensor.matmul(
                                        ps_ew[:],
                                        enct16[:, k, mt * P:(mt + 1) * P],
                                        wcs[:, k, :],
                                        start=(k == 0), stop=(k == KH - 1))
                                nc.vector.tensor_copy(
                                    encwc8[:, mt, n * 256:(n + 1) * 256],
                                    ps_ew[:])

                nc.sync.dma_start(out=whht8[:], in_=whht8_d[:])
                nc.sync.dma_start(out=w1ht8[:], in_=w1ht8_d[:])
                nc.sync.dma_start(out=wo8[:], in_=woutt8_d[:])

                # ---------------- phase 1 + interleaved phase 2 ----------
                p2_queue = []     # pending (j, half, n) vocab chunks
                p2_done = [0] * 4
                fin_at = {20: [0], 28: [1]}

                with (
                    tc.tile_pool(name="p1", bufs=2) as p1,
                    tc.tile_pool(name="p2x", bufs=3) as p2x,
                    tc.tile_pool(name="ps_hp_pool", bufs=1, space="PSUM") as pshp,
                    tc.tile_pool(name="ps_gh_pool", bufs=1, space="PSUM") as psgh,
                    tc.tile_pool(name="ps_gic_pool", bufs=1, space="PSUM") as psgic,
                    tc.tile_pool(name="ps_e_pool", bufs=1, space="PSUM") as pse,
                    tc.tile_pool(name="ps_a_pool", bufs=1, space="PSUM") as psa,
                    tc.tile_pool(name="ps_o_pool", bufs=2, space="PSUM") as pso,
                ):
                    def p2_chunk(j, half, n):
                        w = CW[j]
                        rows = 4 * w
                        nsl = slice(n * NVS, (n + 1) * NVS)
                        ps_o = pso.tile([P, NVS], F32, name="ps_o", tag="ps_o")
                        for k in range(KH):
                            nc.tensor.matmul(
                                ps_o[:],
                                hgat16[:, k, half, j, :, :].rearrange(
                                    "p m c -> p (m c)"),
                                wo8[:, k, nsl],
                                start=(k == 0), stop=(k == KH - 1))
                        lg = lg_t[j][half]
                        nc.vector.scalar_tensor_tensor(
                            lg[:, nsl], ps_o[:], ISW,
                            boutrep[:, nsl], op0=ALU.mult, op1=ALU.add)
                        etr = p2x.tile([P, NVS], F16, name="etr", tag="etr")
                        nc.scalar.activation(
                            etr[:], lg[:, nsl], AF.Exp,
                            accum_out=sums_t[j][half][:, n:n + 1])
                        p2_done[j] += 1
                        if n == NV - 1:
                            ssum = p2x.tile([P, 1], F32, name="ssum",
                                            tag="ssum")
                            nc.vector.reduce_sum(
                                ssum[:], sums_t[j][half][:],
                                axis=mybir.AxisListType.X)
                            nc.sync.dma_start(
                                out=arin[j][half * P:(half + 1) * P, :],
                                in_=ssum[:])
                        if p2_done[j] == 2 * NV:
                            nc.gpsimd.collective_compute(
                                "AllReduce", ALU.add, replica_groups=rg,
                                ins=[arin[j].opt()], outs=[arout[j].opt()])

                    def finalize(j):
                        (tlo, thi) = AG_CHUNKS[j]
                        w = CW[j]
                        for h_ in range(2):
                            lzt = p2x.tile([P, 1], F32, name="lzt", tag="lzt")
                            nc.sync.dma_start(
                                out=lzt[:],
                                in_=arout[j][h_ * P:(h_ + 1) * P, :])
                            lzl = p2x.tile([P, 1], F32, name="lzl", tag="lzl")
                            nc.scalar.activation(lzl[:], lzt[:], AF.Ln)
                            nlz = p2x.tile([P, 1], F32, name="nlz", tag="nlz")
                            nc.vector.tensor_scalar(
                                nlz[:], lzl[:], -1.0, None, op0=ALU.mult)
                            lg = lg_t[j][h_]
                            half_v = VC // 4
                            for vv in range(4):
                                vsl = slice(vv * half_v, (vv + 1) * half_v)
                                ost = p2x.tile([P, half_v], F32, name="ost",
                                               tag="ost", bufs=3)
                                nc.vector.tensor_scalar(
                                    ost[:], lg[:, vsl],
                                    lzl[:, 0:1], None,
                                    op0=ALU.subtract)
                                for m in range(4):
                                    r0 = (h_ * 4 + m) * ROWS + (tlo - 1) * BC
                                    nc.gpsimd.dma_start(
                                        out=out_d[r0:r0 + w, vsl],
                                        in_=ost[m * 32:m * 32 + w, :])

                    for t in range(1, T):
                        hprev = hallT[:, :, t - 1, :]

                        # hproj (fp8 W1h, x16)
                        ps_hp = pshp.tile([P, KH, BC], F32, name="ps_hp",
                                          tag="hp")
                        for mo in range(KH):
                            for k in range(KH):
                                nc.tensor.matmul(
                                    ps_hp[:, mo, :],
                                    w1ht8[:, k, mo * P:(mo + 1) * P],
                                    hprev[:, k, :],
                                    start=(k == 0), stop=(k == KH - 1))

                        # gh (fp8 W_hh, x16)
                        ps_gh = psgh.tile([P, KG, BC], F32, name="ps_gh",
                                          tag="gh")
                        for mo in range(KG):
                            for k in range(KH):
                                nc.tensor.matmul(
                                    ps_gh[:, mo, :],
                                    whht8[:, k, mo * P:(mo + 1) * P],
                                    hprev[:, k, :],
                                    start=(k == 0), stop=(k == KH - 1))

                        # attention: aw = tanh((EcT16 + Hproj)/16), e = w2.aw
                        ps_e = pse.tile([1, NBS], F32, name="ps_e", tag="e")
                        for g in range(4):
                            msl = slice(2 * g, 2 * g + 2)
                            awp = p1.tile([P, 2, BC, SP], F16, name="awp",
                                          tag="awp")
                            nc.vector.tensor_add(
                                awp[:],
                                ps_hp[:, msl, :].broadcast_to([P, 2, BC, SP]),
                                ecT16[:, msl, :].rearrange(
                                    "p m (b s) -> p m b s", s=SP))
                            nc.scalar.activation(
                                aw16[:, msl, :].rearrange(
                                    "p m (b s) -> p m b s", s=SP),
                                awp[:], AF.Tanh, scale=ISW)
                            for mo in range(2 * g, 2 * g + 2):
                                nc.tensor.matmul(
                                    ps_e[:], w2t16[:, mo:mo + 1],
                                    aw16[:, mo, :],
                                    start=(mo == 0), stop=(mo == KH - 1))

                        # softmax over s (no max-shift: |e| small)
                        expe = p1.tile([1, NBS], F32, name="expe", tag="expe")
                        nc.scalar.activation(expe[:], ps_e[:], AF.Exp)
                        s4 = p1.tile([1, BC], F32, name="s4", tag="s4")
                        nc.vector.reduce_sum(
                            s4[:], expe[:].rearrange("a (b s) -> a b s", s=SP)
                            [:, :, 0:S],
                            axis=mybir.AxisListType.X)
                        r4 = p1.tile([1, BC], F32, name="r4", tag="r4")
                        nc.vector.reciprocal(r4[:], s4[:])
                        alphan = p1.tile([1, NBS], F16, name="alphan",
                                         tag="aln")
                        av = alphan[:].rearrange("a (b s) -> a b s", s=SP)
                        nc.vector.tensor_mul(
                            av, expe[:].rearrange("a (b s) -> a b s", s=SP),
                            r4[:].broadcast_to([1, BC, SP]))
                        nc.vector.memset(
                            alphan[:].rearrange(
                                "a (b s) -> a b s", s=SP)[:, :, S:SP], 0.0)

                        # transpose alpha to partitions via K=1 matmuls
                        ps_a = psa.tile([P, 2], F32, name="ps_a", tag="a")
                        nc.tensor.matmul(ps_a[:, 0:1], alphan[:, 0:P],
                                         ones1[:], start=True, stop=True)
                        nc.tensor.matmul(ps_a[:, 1:2], alphan[:, P:NBS],
                                         ones1[:], start=True, stop=True)
                        nc.vector.tensor_copy(bd1[0:64, 0:1], ps_a[0:64, 0:1])
                        nc.vector.tensor_copy(bd1[64:128, 1:2],
                                              ps_a[64:128, 0:1])
                        nc.vector.tensor_copy(bd2[0:64, 2:3], ps_a[0:64, 1:2])
                        nc.vector.tensor_copy(bd2[64:128, 3:4],
                                              ps_a[64:128, 1:2])

                        # gi_c = blockdiag(alpha) applied to EncWc (fp8, x16)
                        ps_gic = psgic.tile([P, KG, BC], F32, name="ps_gic",
                                            tag="gic")
                        for mo in range(KG):
                            nc.tensor.matmul(
                                ps_gic[:, mo, :],
                                encwc8[:, 0, mo * P:(mo + 1) * P],
                                bd1[:], start=True, stop=False)
                            nc.tensor.matmul(
                                ps_gic[:, mo, :],
                                encwc8[:, 1, mo * P:(mo + 1) * P],
                                bd2[:], start=False, stop=True)

                        # gates (all pre-activations carry x16 scale;
                        # sigmoid(x) = 0.5 + 0.5*tanh(x/2) -> Tanh only)
                        s1 = p1.tile([P, KG, BC], F16, name="s1", tag="s1")
                        nc.vector.tensor_add(s1[:], ps_gic[:],
                                             gixt[:, :, t - 1, :])
                        srz = p1.tile([P, 2 * KH, BC], F16, name="srz",
                                      tag="srz")
                        nc.vector.tensor_add(srz[:], s1[:, 0:2 * KH, :],
                                             ps_gh[:, 0:2 * KH, :])
                        rzt = p1.tile([P, 2 * KH, BC], F16, name="rzt",
                                      tag="rzt")
                        nc.scalar.activation(rzt[:], srz[:], AF.Tanh,
                                             scale=0.5 * ISW)
                        rz = p1.tile([P, 2 * KH, BC], F16, name="rz", tag="rz")
                        nc.vector.tensor_scalar(rz[:], rzt[:], 0.5, 0.5,
                                                op0=ALU.mult, op1=ALU.add)
                        hn = p1.tile([P, KH, BC], F16, name="hn", tag="hn")
                        nc.vector.tensor_add(hn[:], ps_gh[:, 2 * KH:KG, :],
                                             bhnrep[:])
                        m1 = p1.tile([P, KH, BC], F16, name="m1", tag="m1")
                        nc.vector.tensor_mul(m1[:], rz[:, 0:KH, :], hn[:])
                        s3 = p1.tile([P, KH, BC], F16, name="s3", tag="s3")
                        nc.vector.tensor_add(s3[:], s1[:, 2 * KH:KG, :], m1[:])
                        nn_t = p1.tile([P, KH, BC], F16, name="nn_t", tag="nn")
                        nc.scalar.activation(nn_t[:], s3[:], AF.Tanh,
                                             scale=ISW)
                        dd = p1.tile([P, KH, BC], F16, name="dd", tag="dd")
                        nc.vector.tensor_sub(dd[:], hprev, nn_t[:])
                        m2 = p1.tile([P, KH, BC], F16, name="m2", tag="m2")
                        nc.vector.tensor_mul(m2[:], rz[:, KH:2 * KH, :], dd[:])
                        nc.vector.tensor_add(hallT[:, :, t, :], nn_t[:], m2[:])

                        # partial allgather of finished h slots (fp16)
                        for j, (tlo, thi) in enumerate(AG_CHUNKS):
                            if t == thi - 1:
                                w = CW[j]
                                for k in range(KH):
                                    nc.sync.dma_start(
                                        out=agin[j][k * P:(k + 1) * P, :]
                                            .rearrange("p (t b) -> p t b",
                                                       b=BC),
                                        in_=hallT[:, k, tlo:thi, :])
                                nc.gpsimd.collective_compute(
                                    "AllGather", ALU.bypass, replica_groups=rg,
                                    ins=[agin[j].opt()], outs=[agout[j].opt()])
                                for k in range(KH):
                                    hgs = p1.tile([P, NCORES, 32], F16,
                                                  name="hgs", tag="hgs")
                                    nc.sync.dma_start(
                                        out=hgs[:, :, 0:w],
                                        in_=agout[j][:, k * P:(k + 1) * P, :]
                                            .rearrange("r p w -> p r w"))
                                    nc.vector.tensor_copy(
                                        hgat16[:, k, :, j, :, 0:w],
                                        hgs[:, :, 0:w].rearrange(
                                            "p (x m) w -> p x m w", x=2))
                                for half in range(2):
                                    for n in range(NV):
                                        p2_queue.append((j, half, n))

                        # paced phase-2 interleave: 2 chunks per step
                        for _ in range(2):
                            if p2_queue:
                                p2_chunk(*p2_queue.pop(0))
                        for j in fin_at.get(t, []):
                            finalize(j)

                    # tail: remaining vocab chunks + finalizes
                    while p2_queue:
                        p2_chunk(*p2_queue.pop(0))
                    finalize(2)
                    finalize(3)

            pwo_cm.__exit__(None, None, None)

    nc.compile()
    return nc


def _t8(w, nk=8):
    # [nk*128, M] -> [128, nk, M]
    m = w.shape[1]
    return np.ascontiguousarray(w.reshape(nk, P, m).transpose(1, 0, 2))


def _f8(x):
    return np.ascontiguousarray(np.asarray(x * SW, dtype=np.float32)).astype(
        ml_dtypes.float8_e4m3)


def _prep_inputs(inputs):
    enc = np.asarray(inputs["encoder_outputs"], np.float32)
    ehid = np.asarray(inputs["encoder_hidden"], np.float32)
    targets = np.asarray(inputs["targets"])
    emb = np.asarray(inputs["emb"], np.float32)
    W1 = np.asarray(inputs["attn_W1"], np.float32)
    b1 = np.asarray(inputs["attn_b1"], np.float32)
    W2 = np.asarray(inputs["attn_W2"], np.float32)
    W_ih = np.asarray(inputs["W_ih"], np.float32)
    b_ih = np.asarray(inputs["b_ih"], np.float32)
    W_hh = np.asarray(inputs["W_hh"], np.float32)
    b_hh = np.asarray(inputs["b_hh"], np.float32)
    W_out = np.asarray(inputs["W_out"], np.float32)
    b_out = np.asarray(inputs["b_out"], np.float32)

    # shared (replicated across cores); all recurrence weights fp8 e4m3 x16
    w1et8 = _f8(_t8(W1[:, :H].T))
    w1ht8 = _f8(_t8(np.ascontiguousarray(W1[:, H:]).T))
    wct8 = _f8(_t8(np.ascontiguousarray(W_ih[:, Dw:]).T))
    whht8 = _f8(_t8(W_hh.T))
    wxa = np.zeros((640, 3 * H), np.float32)
    wxa[:Dw] = W_ih[:, :Dw].T
    wxa[Dw] = b_ih + np.concatenate([b_hh[:2 * H], np.zeros(H, np.float32)])
    wxat8 = _f8(_t8(wxa, nk=5))
    w2t16 = np.ascontiguousarray(W2[0].reshape(KH, P).T).astype(np.float16)
    b1t = np.ascontiguousarray(b1.reshape(KH, P).T) * SW
    bhnrep = np.ascontiguousarray(
        np.repeat(b_hh[2 * H:].reshape(KH, P).T[:, :, None], BC, axis=2)
        .reshape(P, KH * BC)) * SW

    x_all = emb[targets[:, :TS]]  # [B, TS, Dw]

    in_maps = []
    for c in range(NCORES):
        bsl = slice(c * BC, (c + 1) * BC)
        vsl = slice(c * VC, (c + 1) * VC)
        encT = np.zeros((H, BC, SP), np.float32)
        encT[:, :, :S] = enc[bsl].transpose(2, 0, 1)
        enct16 = _t8(encT.reshape(H, NBS)).astype(np.float16)
        xat = np.zeros((640, P), np.float32)
        xat[:Dw, :ROWS] = x_all[bsl].transpose(2, 1, 0).reshape(Dw, ROWS)
        xat[Dw, :ROWS] = 1.0
        xat16 = _t8(xat, nk=5).astype(np.float16)
        h0t = np.ascontiguousarray(
            ehid[0, bsl].T.reshape(KH, P, BC).transpose(1, 0, 2)
            .reshape(P, KH * BC)).astype(np.float16)
        woutt8 = _f8(_t8(np.ascontiguousarray(W_out[vsl]).T))
        bout16 = np.ascontiguousarray(b_out[vsl][None, :]).astype(np.float16)
        in_maps.append({
            "enct16": enct16, "w1et8": w1et8, "wct8": wct8,
            "wxat8": wxat8, "xat16": xat16, "whht8": whht8, "w1ht8": w1ht8,
            "w2t16": w2t16, "b1t": b1t, "bhnrep": bhnrep, "h0t": h0t,
            "woutt8": woutt8, "bout16": bout16,
        })
    return in_maps


def kernel(**inputs):
    if "nc" not in _CACHE:
        _CACHE["nc"] = _build()
    nc = _CACHE["nc"]
    in_maps = _prep_inputs(inputs)
    res = run_bass_kernel_spmd(nc, in_maps, core_ids=list(range(NCORES)))
    L = np.stack([res.results[c]["out"] for c in range(NCORES)])
    L = (L.reshape(NCORES, NCORES, TS, BC, VC)
         .transpose(1, 3, 2, 0, 4).reshape(B, TS, V))
    return np.ascontiguousarray(L, dtype=np.float32)


# revision 30
# speedup vs baseline: 1.1376x; 1.0058x over previous
"""Attention-GRU decoder (teacher forcing) on 8 TRN2 NeuronCores.

v2 strategy (batch-sharded 4 seqs/core, vocab-sharded output):
  Phase 0: precompute EcT (+b1 folded), EncWc, GIX with fp8(x16) weights.
  Phase 1: 31 sequential steps. All recurrence weights (W_hh, W1h, Wc, Wx,
     W1e) are fp8 e4m3 scaled x16 (stationary operands, FWL 4x weight load);
     descale folds into activation `scale` args. sigmoid == 0.5+0.5*tanh(x/2)
     so phase 1 only ever uses Tanh/Exp -> zero act-table thrashing.
     h kept fp16; partial AllGathers (fp16) every 8 steps.
  Phase 2 interleaved: after each partial AllGather, the vocab projection
     (fp8 W_out slice, x16) for those rows runs inside phase-1's tensor
     engine gaps (keeps HAM un-throttled). log-softmax denominators
     AllReduce'd per chunk; finalize (Ln + subtract) deferred/paced.

kernel(**inputs) takes full inputs, returns [B, T-1, V] float32.
"""
import numpy as np
import ml_dtypes

import concourse.bacc as bacc
import concourse.bass as bass
import concourse.bass_utils as _bu
import concourse.mybir as mybir
import concourse.tile as tile
from concourse.bass_utils import run_bass_kernel_spmd

del _bu  # (ldw-opt experiment removed: incompatible with K=1 ldweights)

F32 = mybir.dt.float32
F16 = mybir.dt.float16
F8 = mybir.dt.float8e4
AF = mybir.ActivationFunctionType
ALU = mybir.AluOpType

B, S, H, V, Dw, T = 32, 50, 1024, 32000, 512, 32
NCORES = 8
P = 128
TS = T - 1            # 31 decode steps
BC = B // NCORES      # 4 sequences per core
VC = V // NCORES      # 4000 vocab rows per core
SP = 64               # padded s-block per sequence
NBS = BC * SP         # 256 padded (b,s) columns per core
ROWS = TS * BC        # 124 hidden rows per rank
RTOT = TS * B         # 992 total rows
KH = H // P           # 8 hidden chunks
KG = 3 * H // P       # 24 gate chunks
NV = 8                # vocab n-chunks per core
NVS = VC // NV        # 500
SW = 16.0             # fp8 weight scale
ISW = 1.0 / SW
AG_CHUNKS = [(1, 9), (9, 17), (17, 25), (25, 32)]
CW = [(thi - tlo) * BC for (tlo, thi) in AG_CHUNKS]   # 32,32,32,28

_CACHE = {}


def _build():
    nc = bacc.Bacc("TRN2", target_bir_lowering=False, debug=False,
                   num_devices=NCORES)

    def din(name, shape, dt):
        return nc.dram_tensor(name, shape, dt, kind="ExternalInput").ap()

    enct16_d = din("enct16", [P, KH, NBS], F16)
    w1et8_d = din("w1et8", [P, KH, H], F8)
    wct8_d = din("wct8", [P, KH, 3 * H], F8)
    wxat8_d = din("wxat8", [P, 5, 3 * H], F8)
    xat16_d = din("xat16", [P, 5, P], F16)
    whht8_d = din("whht8", [P, KH, 3 * H], F8)
    w1ht8_d = din("w1ht8", [P, KH, H], F8)
    w2t16_d = din("w2t16", [P, KH], F16)
    b1t_d = din("b1t", [P, KH], F32)
    bhnrep_d = din("bhnrep", [P, KH * BC], F32)
    h0t_d = din("h0t", [P, KH * BC], F16)
    woutt8_d = din("woutt8", [P, KH, VC], F8)
    bout16_d = din("bout16", [1, VC], F16)
    out_d = nc.dram_tensor("out", [RTOT, VC], F32, kind="ExternalOutput").ap()

    rg = [list(range(NCORES))]
    outv = out_d.rearrange("(m tb) v -> m tb v", tb=ROWS)

    with tile.TileContext(nc) as tc:
        with tc.tile_pool(name="dram", bufs=1, space="DRAM") as dram:
            agin, agout, arin, arout = [], [], [], []
            for j, (tlo, thi) in enumerate(AG_CHUNKS):
                w = CW[j]
                agin.append(dram.tile([H, w], F16, name=f"agin{j}"))
                agout.append(dram.tile([NCORES, H, w], F16, name=f"agout{j}"))
                arin.append(dram.tile([2 * P, 1], F32, name=f"arin{j}"))
                arout.append(dram.tile([2 * P, 1], F32, name=f"arout{j}"))

            pwo_cm = tc.tile_pool(name="pwo", bufs=1)
            pwo = pwo_cm.__enter__()
            wo8 = pwo.tile([P, KH, VC], F8)
            hgat16 = pwo.tile([P, KH, 2, 4, 4, 32], F16)
            boutrep = pwo.tile([P, VC], F16)
            lg_t = [[pwo.tile([P, VC], F16, name=f"lg{j}_{h_}")
                     for h_ in range(2)] for j in range(4)]
            sums_t = [[pwo.tile([P, NV], F32, name=f"sums{j}_{h_}")
                       for h_ in range(2)] for j in range(4)]

            with tc.tile_pool(name="pw", bufs=1) as pw:
                # ---- persistent phase-0/1 tiles ----
                whht8 = pw.tile([P, KH, 3 * H], F8)
                w1ht8 = pw.tile([P, KH, H], F8)
                ecT16 = pw.tile([P, KH, NBS], F16)
                encwc8 = pw.tile([P, 2, 3 * H], F8)
                gixt = pw.tile([P, KG, TS, BC], F16)
                hallT = pw.tile([P, KH, T, BC], F16)
                aw16 = pw.tile([P, KH, NBS], F16)
                w2t16 = pw.tile([P, KH], F16)
                bhnrep = pw.tile([P, KH, BC], F32)
                ones1 = pw.tile([1, 1], F16)
                bd1 = pw.tile([P, BC], F16)
                bd2 = pw.tile([P, BC], F16)

                nc.sync.dma_start(out=w2t16[:], in_=w2t16_d[:])
                nc.sync.dma_start(
                    out=bhnrep[:],
                    in_=bhnrep_d[:].rearrange("p (k b) -> p k b", b=BC))
                nc.sync.dma_start(
                    out=hallT[:, :, 0, :],
                    in_=h0t_d[:].rearrange("p (k b) -> p k b", b=BC))
                nc.vector.memset(ones1[:], 1.0)
                nc.vector.memset(bd1[:], 0.0)
                nc.vector.memset(bd2[:], 0.0)
                nc.vector.memset(aw16[:], 0.0)
                for k in range(KH):
                    nc.vector.memset(hgat16[:, k, :, 3, :, 28:32], 0.0)

                # ---------------- phase 0 ----------------
                with (
                    tc.tile_pool(name="p0a", bufs=1) as p0a,
                    tc.tile_pool(name="p0as", bufs=2) as p0as,
                    tc.tile_pool(name="ps_gx_pool", bufs=1, space="PSUM") as psgx,
                    tc.tile_pool(name="ps_bo_pool", bufs=1, space="PSUM") as psbo,
                ):
                    # b_out broadcast to all partitions (needed in phase 1)
                    ones16 = p0a.tile([1, P], F16)
                    nc.vector.memset(ones16[:], 1.0)
                    bout16 = p0a.tile([1, VC], F16)
                    nc.sync.dma_start(out=bout16[:], in_=bout16_d[:])
                    for n in range(NV):
                        bsl = slice(n * NVS, (n + 1) * NVS)
                        ps_b = psbo.tile([P, NVS], F32, name="ps_b", tag="ps_b")
                        nc.tensor.matmul(ps_b[:], ones16[:], bout16[:, bsl],
                                         start=True, stop=True)
                        nc.scalar.copy(boutrep[:, bsl], ps_b[:])

                    xat16 = p0a.tile([P, 5, P], F16)
                    nc.sync.dma_start(out=xat16[:], in_=xat16_d[:])
                    ps_gx = [psgx.tile([P, 4, P], F32, name=f"ps_gx{g}")
                             for g in range(6)]
                    for k in range(5):
                        wxk = p0as.tile([P, 3 * H], F8, name="wxk", tag="wxk")
                        nc.sync.dma_start(out=wxk[:], in_=wxat8_d[:, k, :])
                        for mo in range(KG):
                            nc.tensor.matmul(
                                ps_gx[mo // 4][:, mo % 4, :],
                                wxk[:, mo * P:(mo + 1) * P],
                                xat16[:, k, :], start=(k == 0), stop=(k == 4))
                    for mo in range(KG):
                        nc.scalar.copy(
                            gixt[:, mo, :, :],
                            ps_gx[mo // 4][:, mo % 4, 0:ROWS].rearrange(
                                "p (t b) -> p t b", b=BC))

                with (
                    tc.tile_pool(name="p0b", bufs=1) as p0b,
                    tc.tile_pool(name="p0bs", bufs=2) as p0bs,
                ):
                    enct16 = p0b.tile([P, KH, NBS], F16)
                    b1t = p0b.tile([P, KH], F32)
                    nc.sync.dma_start(out=b1t[:], in_=b1t_d[:])
                    nc.sync.dma_start(out=enct16[:], in_=enct16_d[:])

                    # EcT (+ b1 folded), fp16 out
                    with tc.tile_pool(name="ps_ec_pool", bufs=1,
                                      space="PSUM") as psec:
                        ps_ec = [psec.tile([P, NBS], F32, name=f"ps_ec{mo}")
                                 for mo in range(KH)]
                        for k in range(KH):
                            w1ek = p0bs.tile([P, H], F8, name="w1ek", tag="w1ek")
                            nc.sync.dma_start(out=w1ek[:], in_=w1et8_d[:, k, :])
                            for mo in range(KH):
                                nc.tensor.matmul(
                                    ps_ec[mo][:], w1ek[:, mo * P:(mo + 1) * P],
                                    enct16[:, k, :],
                                    start=(k == 0), stop=(k == KH - 1))
                        for mo in range(KH):
                            nc.vector.tensor_scalar(
                                ecT16[:, mo, :], ps_ec[mo][:],
                                b1t[:, mo:mo + 1], None, op0=ALU.add)

                    # EncWc, fp8 out (x16 scale via host-scaled wct8)
                    with tc.tile_pool(name="ps_ew_pool", bufs=2,
                                      space="PSUM") as psew:
                        for n in range(12):
                            wcs = p0bs.tile([P, KH, 256], F8, name="wcs",
                                            tag="wcs")
                            nc.sync.dma_start(
                                out=wcs[:],
                                in_=wct8_d[:, :, n * 256:(n + 1) * 256])
                            for mt in range(2):
                                ps_ew = psew.tile([P, 256], F32, name="ps_ew",
                                                  tag="ps_ew")
                                for k in range(KH):
                                    nc.tensor.matmul(
                                        ps_ew[:],
                                        enct16[:, k, mt * P:(mt + 1) * P],
                                        wcs[:, k, :],
                                        start=(k == 0), stop=(k == KH - 1))
                                nc.vector.tensor_copy(
                                    encwc8[:, mt, n * 256:(n + 1) * 256],
                                    ps_ew[:])

                nc.sync.dma_start(out=whht8[:], in_=whht8_d[:])
                nc.sync.dma_start(out=w1ht8[:], in_=w1ht8_d[:])
                nc.sync.dma_start(out=wo8[:], in_=woutt8_d[:])

                # ---------------- phase 1 + interleaved phase 2 ----------
                p2_queue = []     # pending (j, half, n) vocab chunks
                p2_done = [0] * 4
                fin_at = {20: [0], 28: [1]}

                with (
                    tc.tile_pool(name="p1", bufs=2) as p1,
                    tc.tile_pool(name="p2x", bufs=3) as p2x,
                    tc.tile_pool(name="ps_hp_pool", bufs=1, space="PSUM") as pshp,
                    tc.tile_pool(name="ps_gh_pool", bufs=1, space="PSUM") as psgh,
                    tc.tile_pool(name="ps_gic_pool", bufs=1, space="PSUM") as psgic,
                    tc.tile_pool(name="ps_e_pool", bufs=1, space="PSUM") as pse,
                    tc.tile_pool(name="ps_a_pool", bufs=1, space="PSUM") as psa,
                    tc.tile_pool(name="ps_o_pool", bufs=2, space="PSUM") as pso,
                ):
                    def p2_chunk(j, half, n):
                        w = CW[j]
                        rows = 4 * w
                        nsl = slice(n * NVS, (n + 1) * NVS)
                        ps_o = pso.tile([P, NVS], F32, name="ps_o", tag="ps_o")
                        for k in range(KH):
                            nc.tensor.matmul(
                                ps_o[:],
                                hgat16[:, k, half, j, :, :].rearrange(
                                    "p m c -> p (m c)"),
                                wo8[:, k, nsl],
                                start=(k == 0), stop=(k == KH - 1))
                        lg = lg_t[j][half]
                        nc.vector.scalar_tensor_tensor(
                            lg[:, nsl], ps_o[:], ISW,
                            boutrep[:, nsl], op0=ALU.mult, op1=ALU.add)
                        etr = p2x.tile([P, NVS], F16, name="etr", tag="etr")
                        nc.scalar.activation(
                            etr[:], lg[:, nsl], AF.Exp,
                            accum_out=sums_t[j][half][:, n:n + 1])
                        p2_done[j] += 1
                        if n == NV - 1:
                            ssum = p2x.tile([P, 1], F32, name="ssum",
                                            tag="ssum")
                            nc.vector.reduce_sum(
                                ssum[:], sums_t[j][half][:],
                                axis=mybir.AxisListType.X)
                            nc.sync.dma_start(
                                out=arin[j][half * P:(half + 1) * P, :],
                                in_=ssum[:])
                        if p2_done[j] == 2 * NV:
                            nc.gpsimd.collective_compute(
                                "AllReduce", ALU.add, replica_groups=rg,
                                ins=[arin[j].opt()], outs=[arout[j].opt()])

                    def finalize(j):
                        (tlo, thi) = AG_CHUNKS[j]
                        w = CW[j]
                        for h_ in range(2):
                            lzt = p2x.tile([P, 1], F32, name="lzt", tag="lzt")
                            nc.sync.dma_start(
                                out=lzt[:],
                                in_=arout[j][h_ * P:(h_ + 1) * P, :])
                            lzl = p2x.tile([P, 1], F32, name="lzl", tag="lzl")
                            nc.scalar.activation(lzl[:], lzt[:], AF.Ln)
                            nlz = p2x.tile([P, 1], F32, name="nlz", tag="nlz")
                            nc.vector.tensor_scalar(
                                nlz[:], lzl[:], -1.0, None, op0=ALU.mult)
                            lg = lg_t[j][h_]
                            half_v = VC // 4
                            for vv in range(4):
                                vsl = slice(vv * half_v, (vv + 1) * half_v)
                                ost = p2x.tile([P, half_v], F32, name="ost",
                                               tag="ost", bufs=3)
                                nc.vector.tensor_scalar(
                                    ost[:], lg[:, vsl],
                                    lzl[:, 0:1], None,
                                    op0=ALU.subtract)
                                for m in range(4):
                                    r0 = (h_ * 4 + m) * ROWS + (tlo - 1) * BC
                                    nc.gpsimd.dma_start(
                                        out=out_d[r0:r0 + w, vsl],
                                        in_=ost[m * 32:m * 32 + w, :])

                    for t in range(1, T):
                        hprev = hallT[:, :, t - 1, :]

                        # hproj (fp8 W1h, x16)
                        ps_hp = pshp.tile([P, KH, BC], F32, name="ps_hp",
                                          tag="hp")
                        for mo in range(KH):
                            for k in range(KH):
                                nc.tensor.matmul(
                                    ps_hp[:, mo, :],
                                    w1ht8[:, k, mo * P:(mo + 1) * P],
                                    hprev[:, k, :],
                                    start=(k == 0), stop=(k == KH - 1))

                        # gh (fp8 W_hh, x16)
                        ps_gh = psgh.tile([P, KG, BC], F32, name="ps_gh",
                                          tag="gh")
                        for mo in range(KG):
                            for k in range(KH):
                                nc.tensor.matmul(
                                    ps_gh[:, mo, :],
                                    whht8[:, k, mo * P:(mo + 1) * P],
                                    hprev[:, k, :],
                                    start=(k == 0), stop=(k == KH - 1))

                        # attention: aw = tanh((EcT16 + Hproj)/16), e = w2.aw
                        ps_e = pse.tile([1, NBS], F32, name="ps_e", tag="e")
                        for g in range(4):
                            msl = slice(2 * g, 2 * g + 2)
                            awp = p1.tile([P, 2, BC, SP], F16, name="awp",
                                          tag="awp")
                            nc.vector.tensor_add(
                                awp[:],
                                ps_hp[:, msl, :].broadcast_to([P, 2, BC, SP]),
                                ecT16[:, msl, :].rearrange(
                                    "p m (b s) -> p m b s", s=SP))
                            nc.scalar.activation(
                                aw16[:, msl, :].rearrange(
                                    "p m (b s) -> p m b s", s=SP),
                                awp[:], AF.Tanh, scale=ISW)
                            for mo in range(2 * g, 2 * g + 2):
                                nc.tensor.matmul(
                                    ps_e[:], w2t16[:, mo:mo + 1],
                                    aw16[:, mo, :],
                                    start=(mo == 0), stop=(mo == KH - 1))

                        # feed the PE a vocab chunk during the softmax
                        # section (it has no step matmuls to run)
                        if p2_queue:
                            p2_chunk(*p2_queue.pop(0))

                        # softmax over s (no max-shift: |e| small)
                        expe = p1.tile([1, NBS], F32, name="expe", tag="expe")
                        nc.scalar.activation(expe[:], ps_e[:], AF.Exp)
                        s4 = p1.tile([1, BC], F32, name="s4", tag="s4")
                        nc.vector.reduce_sum(
                            s4[:], expe[:].rearrange("a (b s) -> a b s", s=SP)
                            [:, :, 0:S],
                            axis=mybir.AxisListType.X)
                        r4 = p1.tile([1, BC], F32, name="r4", tag="r4")
                        nc.vector.reciprocal(r4[:], s4[:])
                        alphan = p1.tile([1, NBS], F16, name="alphan",
                                         tag="aln")
                        av = alphan[:].rearrange("a (b s) -> a b s", s=SP)
                        nc.vector.tensor_mul(
                            av, expe[:].rearrange("a (b s) -> a b s", s=SP),
                            r4[:].broadcast_to([1, BC, SP]))
                        nc.vector.memset(
                            alphan[:].rearrange(
                                "a (b s) -> a b s", s=SP)[:, :, S:SP], 0.0)

                        # transpose alpha to partitions via K=1 matmuls
                        ps_a = psa.tile([P, 2], F32, name="ps_a", tag="a")
                        nc.tensor.matmul(ps_a[:, 0:1], alphan[:, 0:P],
                                         ones1[:], start=True, stop=True)
                        nc.tensor.matmul(ps_a[:, 1:2], alphan[:, P:NBS],
                                         ones1[:], start=True, stop=True)
                        nc.vector.tensor_copy(bd1[0:64, 0:1], ps_a[0:64, 0:1])
                        nc.vector.tensor_copy(bd1[64:128, 1:2],
                                              ps_a[64:128, 0:1])
                        nc.vector.tensor_copy(bd2[0:64, 2:3], ps_a[0:64, 1:2])
                        nc.vector.tensor_copy(bd2[64:128, 3:4],
                                              ps_a[64:128, 1:2])

                        # gi_c = blockdiag(alpha) applied to EncWc (fp8, x16)
                        ps_gic = psgic.tile([P, KG, BC], F32, name="ps_gic",
                                            tag="gic")
                        for mo in range(KG):
                            nc.tensor.matmul(
                                ps_gic[:, mo, :],
                                encwc8[:, 0, mo * P:(mo + 1) * P],
                                bd1[:], start=True, stop=False)
                            nc.tensor.matmul(
                                ps_gic[:, mo, :],
                                encwc8[:, 1, mo * P:(mo + 1) * P],
                                bd2[:], start=False, stop=True)

                        # gates (all pre-activations carry x16 scale;
                        # sigmoid(x) = 0.5 + 0.5*tanh(x/2) -> Tanh only)
                        s1 = p1.tile([P, KG, BC], F16, name="s1", tag="s1")
                        nc.vector.tensor_add(s1[:], ps_gic[:],
                                             gixt[:, :, t - 1, :])
                        srz = p1.tile([P, 2 * KH, BC], F16, name="srz",
                                      tag="srz")
                        nc.vector.tensor_add(srz[:], s1[:, 0:2 * KH, :],
                                             ps_gh[:, 0:2 * KH, :])
                        rzt = p1.tile([P, 2 * KH, BC], F16, name="rzt",
                                      tag="rzt")
                        nc.scalar.activation(rzt[:], srz[:], AF.Tanh,
                                             scale=0.5 * ISW)
                        rz = p1.tile([P, 2 * KH, BC], F16, name="rz", tag="rz")
                        nc.vector.tensor_scalar(rz[:], rzt[:], 0.5, 0.5,
                                                op0=ALU.mult, op1=ALU.add)
                        hn = p1.tile([P, KH, BC], F16, name="hn", tag="hn")
                        nc.vector.tensor_add(hn[:], ps_gh[:, 2 * KH:KG, :],
                                             bhnrep[:])
                        m1 = p1.tile([P, KH, BC], F16, name="m1", tag="m1")
                        nc.vector.tensor_mul(m1[:], rz[:, 0:KH, :], hn[:])
                        s3 = p1.tile([P, KH, BC], F16, name="s3", tag="s3")
                        nc.vector.tensor_add(s3[:], s1[:, 2 * KH:KG, :], m1[:])
                        nn_t = p1.tile([P, KH, BC], F16, name="nn_t", tag="nn")
                        nc.scalar.activation(nn_t[:], s3[:], AF.Tanh,
                                             scale=ISW)
                        dd = p1.tile([P, KH, BC], F16, name="dd", tag="dd")
                        nc.vector.tensor_sub(dd[:], hprev, nn_t[:])
                        m2 = p1.tile([P, KH, BC], F16, name="m2", tag="m2")
                        nc.vector.tensor_mul(m2[:], rz[:, KH:2 * KH, :], dd[:])
                        nc.vector.tensor_add(hallT[:, :, t, :], nn_t[:], m2[:])

                        # partial allgather of finished h slots (fp16)
                        for j, (tlo, thi) in enumerate(AG_CHUNKS):
                            if t == thi - 1:
                                w = CW[j]
                                for k in range(KH):
                                    nc.sync.dma_start(
                                        out=agin[j][k * P:(k + 1) * P, :]
                                            .rearrange("p (t b) -> p t b",
                                                       b=BC),
                                        in_=hallT[:, k, tlo:thi, :])
                                nc.gpsimd.collective_compute(
                                    "AllGather", ALU.bypass, replica_groups=rg,
                                    ins=[agin[j].opt()], outs=[agout[j].opt()])
                                for k in range(KH):
                                    hgs = p1.tile([P, NCORES, 32], F16,
                                                  name="hgs", tag="hgs")
                                    nc.sync.dma_start(
                                        out=hgs[:, :, 0:w],
                                        in_=agout[j][:, k * P:(k + 1) * P, :]
                                            .rearrange("r p w -> p r w"))
                                    nc.vector.tensor_copy(
                                        hgat16[:, k, :, j, :, 0:w],
                                        hgs[:, :, 0:w].rearrange(
                                            "p (x m) w -> p x m w", x=2))
                                for half in range(2):
                                    for n in range(NV):
                                        p2_queue.append((j, half, n))

                        # paced phase-2 interleave: 1 more chunk per step
                        if p2_queue:
                            p2_chunk(*p2_queue.pop(0))
                        for j in fin_at.get(t, []):
                            finalize(j)

                    # tail: remaining vocab chunks + finalizes
                    while p2_queue:
                        p2_chunk(*p2_queue.pop(0))
                    finalize(2)
                    finalize(3)

            pwo_cm.__exit__(None, None, None)

    nc.compile()
    return nc


def _t8(w, nk=8):
    # [nk*128, M] -> [128, nk, M]
    m = w.shape[1]
    return np.ascontiguousarray(w.reshape(nk, P, m).transpose(1, 0, 2))


def _f8(x):
    return np.ascontiguousarray(np.asarray(x * SW, dtype=np.float32)).astype(
        ml_dtypes.float8_e4m3)


def _prep_inputs(inputs):
    enc = np.asarray(inputs["encoder_outputs"], np.float32)
    ehid = np.asarray(inputs["encoder_hidden"], np.float32)
    targets = np.asarray(inputs["targets"])
    emb = np.asarray(inputs["emb"], np.float32)
    W1 = np.asarray(inputs["attn_W1"], np.float32)
    b1 = np.asarray(inputs["attn_b1"], np.float32)
    W2 = np.asarray(inputs["attn_W2"], np.float32)
    W_ih = np.asarray(inputs["W_ih"], np.float32)
    b_ih = np.asarray(inputs["b_ih"], np.float32)
    W_hh = np.asarray(inputs["W_hh"], np.float32)
    b_hh = np.asarray(inputs["b_hh"], np.float32)
    W_out = np.asarray(inputs["W_out"], np.float32)
    b_out = np.asarray(inputs["b_out"], np.float32)

    # shared (replicated across cores); all recurrence weights fp8 e4m3 x16
    w1et8 = _f8(_t8(W1[:, :H].T))
    w1ht8 = _f8(_t8(np.ascontiguousarray(W1[:, H:]).T))
    wct8 = _f8(_t8(np.ascontiguousarray(W_ih[:, Dw:]).T))
    whht8 = _f8(_t8(W_hh.T))
    wxa = np.zeros((640, 3 * H), np.float32)
    wxa[:Dw] = W_ih[:, :Dw].T
    wxa[Dw] = b_ih + np.concatenate([b_hh[:2 * H], np.zeros(H, np.float32)])
    wxat8 = _f8(_t8(wxa, nk=5))
    w2t16 = np.ascontiguousarray(W2[0].reshape(KH, P).T).astype(np.float16)
    b1t = np.ascontiguousarray(b1.reshape(KH, P).T) * SW
    bhnrep = np.ascontiguousarray(
        np.repeat(b_hh[2 * H:].reshape(KH, P).T[:, :, None], BC, axis=2)
        .reshape(P, KH * BC)) * SW

    x_all = emb[targets[:, :TS]]  # [B, TS, Dw]

    in_maps = []
    for c in range(NCORES):
        bsl = slice(c * BC, (c + 1) * BC)
        vsl = slice(c * VC, (c + 1) * VC)
        encT = np.zeros((H, BC, SP), np.float32)
        encT[:, :, :S] = enc[bsl].transpose(2, 0, 1)
        enct16 = _t8(encT.reshape(H, NBS)).astype(np.float16)
        xat = np.zeros((640, P), np.float32)
        xat[:Dw, :ROWS] = x_all[bsl].transpose(2, 1, 0).reshape(Dw, ROWS)
        xat[Dw, :ROWS] = 1.0
        xat16 = _t8(xat, nk=5).astype(np.float16)
        h0t = np.ascontiguousarray(
            ehid[0, bsl].T.reshape(KH, P, BC).transpose(1, 0, 2)
            .reshape(P, KH * BC)).astype(np.float16)
        woutt8 = _f8(_t8(np.ascontiguousarray(W_out[vsl]).T))
        bout16 = np.ascontiguousarray(b_out[vsl][None, :]).astype(np.float16)
        in_maps.append({
            "enct16": enct16, "w1et8": w1et8, "wct8": wct8,
            "wxat8": wxat8, "xat16": xat16, "whht8": whht8, "w1ht8": w1ht8,
            "w2t16": w2t16, "b1t": b1t, "bhnrep": bhnrep, "h0t": h0t,
            "woutt8": woutt8, "bout16": bout16,
        })
    return in_maps


def kernel(**inputs):
    if "nc" not in _CACHE:
        _CACHE["nc"] = _build()
    nc = _CACHE["nc"]
    in_maps = _prep_inputs(inputs)
    res = run_bass_kernel_spmd(nc, in_maps, core_ids=list(range(NCORES)))
    L = np.stack([res.results[c]["out"] for c in range(NCORES)])
    L = (L.reshape(NCORES, NCORES, TS, BC, VC)
         .transpose(1, 3, 2, 0, 4).reshape(B, TS, V))
    return np.ascontiguousarray(L, dtype=np.float32)
